# revision 10
# baseline (speedup 1.0000x reference)
"""ExtractTensorPatches kernel for 8 trn2 NeuronCores.

Problem: x (4, 32, 256, 256) f32 -> out (4, 961, 32, 16, 16) f32 with
  out[b, ho*31+wo, c, i, j] = x[b, c, 8*ho+i, 8*wo+j] + EPS * patchsum
  patchsum = sum over the 16x16 patch at (8*ho, 8*wo).

Sharding: pure data parallelism over channels. Core k handles channels
[4k, 4k+4) for all 4 batches. Host gathers with concat on axis 2.

Per-core layout (one tile set per batch b):
  X tile  [124, 4096]: partition p=(hp, c) (hp=band 0..30, c=0..3); the
          partition holds rows 8*hp..8*hp+15 (16 rows x 256 cols) of
          channel c. Adjacent bands overlap by 8 rows -> 2x read amp,
          but every DMA run is 4KB+ contiguous DRAM.
  R1      [124, 512]: per-(row i, 8-col-block k) partial sums.
  S       [124, 31]:  per-(band, wo) 16x16 patch sums.
  OUT     [124, 7936]: free = (wo, i, j); computed in ONE fused DVE op
          out = (S * EPS) + X_widened, where X is read with the
          overlapping window AP (wo stride 8, window 16).
  OUT DMA: partition (hp, c) -> out[b, hp*31+wo, c, i, j]; 1KB runs.
"""

import sys

for _p in ("/opt/trn_rl_repo", "/root/.axon_site/_ro/trn_rl_repo"):
    if _p not in sys.path:
        sys.path.append(_p)

import numpy as np

B, C, H, W = 4, 32, 256, 256
WIN, STR = 16, 8
HO = (H - WIN) // STR + 1  # 31
L = HO * HO  # 961
EPS = 1e-6
NCORES = 8
CLOC = C // NCORES  # 4 channels per core
NP_PART = HO * CLOC  # 124 partitions in use

_nc_cache = {}


def _mk(t, dims):
    """Build a custom AP on a pool tile: partition dim + given free dims."""
    import concourse.bass as bass

    pstep = 1
    for d in t.tensor.shape[1:]:
        pstep *= d
    return bass.AP(t.tensor, t.offset, [[pstep, t.shape[0]]] + [list(d) for d in dims])


def build_nc():
    import concourse.bacc as bacc
    import concourse.mybir as mybir
    import concourse.tile as tile

    f32 = mybir.dt.float32
    nc = bacc.Bacc(
        "TRN2", target_bir_lowering=False, debug=False, num_devices=NCORES
    )
    x = nc.dram_tensor("x", [B, CLOC, H, W], f32, kind="ExternalInput").ap()
    # per-core layout (B, C_loc, ho, wo, i, j): each SBUF partition's
    # store is one fully-contiguous 31744B DRAM chunk (host permutes
    # back to (B, L, C, i, j) during the unshard gather).
    out = nc.dram_tensor(
        "out", [B, CLOC, HO, HO, WIN, WIN], f32, kind="ExternalOutput"
    ).ap()
    import concourse.bass as bass

    with tile.TileContext(nc) as tc:
        with (
            tc.tile_pool(name="xin", bufs=2) as xpool,
            tc.tile_pool(name="stats", bufs=2) as spool,
            tc.tile_pool(name="outp", bufs=2) as opool,
        ):
            for b in range(B):
                # ---- load: partition (c, hp) <- rows 8hp..8hp+15 of chan c
                X = xpool.tile([NP_PART, WIN * W], f32, tag="X")
                src = bass.AP(
                    x.tensor,
                    b * CLOC * H * W,
                    [[H * W, CLOC], [STR * W, HO], [1, WIN * W]],
                )
                nc.gpsimd.dma_start(out=_mk(X, [[1, WIN * W]]), in_=src)

                # ---- R1[p, i*32+k] = sum_{j8} X[p, i*256 + 8k + j8]
                R1 = spool.tile([NP_PART, WIN * 32], f32, tag="R1")
                nc.vector.reduce_sum(
                    out=_mk(R1, [[1, WIN * 32]]),
                    in_=_mk(X, [[W, WIN], [8, 32], [1, 8]]),
                    axis=mybir.AxisListType.X,
                )
                # ---- S[p, wo] = sum_{i, d in {0,1}} R1[p, i*32 + wo + d]
                S = spool.tile([NP_PART, HO], f32, tag="S")
                nc.vector.reduce_sum(
                    out=_mk(S, [[1, HO]]),
                    in_=_mk(R1, [[1, HO], [32, WIN], [1, 2]]),
                    axis=mybir.AxisListType.XY,
                )

                # ---- OUT[p, (wo,i,j)] = (S[p,wo] * EPS) + X[p, i*256+8wo+j]
                # walrus requires <=3D stt inputs -> one op per patch row i.
                OUT = opool.tile([NP_PART, HO * WIN * WIN], f32, tag="OUT")
                opstep = 1
                for d in OUT.tensor.shape[1:]:
                    opstep *= d
                xpstep = 1
                for d in X.tensor.shape[1:]:
                    xpstep *= d
                for i in range(WIN):
                    out_ap = bass.AP(
                        OUT.tensor,
                        OUT.offset + i * WIN,
                        [[opstep, NP_PART], [WIN * WIN, HO], [1, WIN]],
                    )
                    in1_ap = bass.AP(
                        X.tensor,
                        X.offset + i * W,
                        [[xpstep, NP_PART], [STR, HO], [1, WIN]],
                    )
                    nc.vector.scalar_tensor_tensor(
                        out=out_ap,
                        in0=_mk(S, [[1, HO], [0, WIN]]),
                        scalar=float(EPS),
                        in1=in1_ap,
                        op0=mybir.AluOpType.mult,
                        op1=mybir.AluOpType.add,
                    )

                # ---- store: partition (c, hp) -> out[b, c, hp, :, :, :]
                # One SWDGE DMA per batch: per-partition-contiguous DRAM,
                # round-robined across all 16 SDMA engines.
                fpp = HO * WIN * WIN  # 7936 elems per partition
                dst = bass.AP(
                    out.tensor,
                    b * CLOC * HO * fpp,
                    [[HO * fpp, CLOC], [fpp, HO], [1, fpp]],
                )
                nc.gpsimd.dma_start(out=dst, in_=OUT[:, :])

    nc.compile()
    return nc


def get_nc():
    if "nc" not in _nc_cache:
        _nc_cache["nc"] = build_nc()
    return _nc_cache["nc"]


def kernel(x: np.ndarray) -> np.ndarray:
    from concourse.bass_utils import run_bass_kernel_spmd

    x = np.ascontiguousarray(np.asarray(x, dtype=np.float32))
    nc = get_nc()
    in_maps = [
        {"x": np.ascontiguousarray(x[:, k * CLOC : (k + 1) * CLOC])}
        for k in range(NCORES)
    ]
    res = run_bass_kernel_spmd(nc, in_maps, list(range(NCORES)))
    # res[k]["out"]: (B, CLOC, ho, wo, i, j) -> full (B, L, C, i, j)
    arr = np.stack([r["out"] for r in res.results], axis=0)
    return np.ascontiguousarray(
        arr.transpose(1, 3, 4, 0, 2, 5, 6).reshape(B, L, C, WIN, WIN)
    )
